# revision 11
# baseline (speedup 1.0000x reference)
"""Grouped MoE MLP (SwiGLU) for TRN2, expert-parallel across 8 NeuronCores.

Problem: T=8192 tokens pre-permuted into 8 contiguous expert segments of 1024,
H=1024, I=2816, per-expert weights gate/up [H,I], down [I,H].
    o1 = x @ gate; o2 = x @ up; h = silu(o1)*o2; out = h @ down

Sharding: expert-parallel - core e computes expert e's segment entirely
(zero collectives). Host slices inputs per expert and concatenates outputs.

Device kernel (per core), all matmuls bf16 (rel err ~4e-3, budget 2e-2):
  - Startup is DMA-delivery-bound: queues go live ~8.5us (after the fixed
    ~7us NEFF prologue) at ~130/130/80 GB/s (scalar/sync/gpsimd). Wave-1
    interleaves slab-0 weights + x tiles across all three queues in
    consumption order; the PE p-state ramp (0.65/1.2GHz for the first
    ~3us of a busy streak) softens the early delivery deadlines.
  - stage 1 (one pass, slab period 6.9us): per i-slab, o1T/o2T [128i,512]
    PSUM-accumulate over 8 h-chunks per 512-token chunk; SwiGLU fused
    scalar(silu)+vector(mul) into resident hT bf16 [I, TE].
  - Weight DMAs (g half on sync, u half on scalar) are emitted inside
    slab bodies at prefetch distance 2, so no queue builds a deep
    backlog; down-proj tiles stream on sync 3-per-slab from slab 4.
    PE stalls are poison: any gap drops the PE out of max p-state for
    ~3us of half-rate matmuls. ps1 bufs=3 decouples the SwiGLU readers
    from PSUM slot reuse.
  - stage 2: out[m,hc] = sum_i hT_i[:,m].T @ down_i[:,hc], 22-long PSUM
    chains; output stored bf16 (host upcasts); last tile split in half
    to shorten the final flush tail.
"""

import os
import numpy as np
from contextlib import ExitStack

E, H, I, T = 8, 1024, 2816, 8192
TE = T // E  # tokens per expert = 1024
KC = H // 128  # 8 h-chunks
IS = I // 128  # 22 i-slabs
NCH = 512  # moving free dim per matmul (one PSUM bank of fp32)

_cache: dict = {}


def _build_nc(dt_tag: str):
    from concourse import bacc
    import concourse.tile as tile
    import concourse.mybir as mybir
    from concourse.bass import ts

    f32 = mybir.dt.float32
    dt = {"f32r": mybir.dt.float32r, "bf16": mybir.dt.bfloat16}[dt_tag]

    nc = bacc.Bacc("TRN2", target_bir_lowering=False, debug=False, num_devices=8)
    xt_d = nc.dram_tensor("xt", [2, 128, KC, NCH], dt, kind="ExternalInput").ap()
    gu_d = nc.dram_tensor("gu", [IS, 128, 2, KC, 128], dt, kind="ExternalInput").ap()
    down_d = nc.dram_tensor("down", [IS, 128, H], dt, kind="ExternalInput").ap()
    out_d = nc.dram_tensor("out", [TE, H], dt, kind="ExternalOutput").ap()

    silu_fn = mybir.ActivationFunctionType.Silu

    with tile.TileContext(nc) as tc, ExitStack() as ctx:
        xt_pool = ctx.enter_context(tc.tile_pool(name="xt", bufs=1))
        gu_pool = ctx.enter_context(tc.tile_pool(name="gu", bufs=4))
        h_pool = ctx.enter_context(tc.tile_pool(name="h", bufs=IS))
        d_pool = ctx.enter_context(tc.tile_pool(name="d", bufs=2 * IS))
        s_pool = ctx.enter_context(tc.tile_pool(name="s", bufs=3))
        o_pool = ctx.enter_context(tc.tile_pool(name="o", bufs=3))
        ps1 = ctx.enter_context(tc.tile_pool(name="ps1", bufs=3, space="PSUM"))
        ps3 = ctx.enter_context(tc.tile_pool(name="ps3", bufs=2, space="PSUM"))

        # resident x chunks; gu slabs are double+ buffered (prefetch dist 2)
        xtall = [
            xt_pool.tile([128, KC, NCH], dt, tag=f"xt{tci}", name=f"xt{tci}", bufs=1)
            for tci in range(2)
        ]
        gus = {0: gu_pool.tile([128, 2, KC, 128], dt, tag="gu", name="gu0")}
        g0 = gus[0]

        # ---- wave 1: slab-0 weights + x, interleaved across queues in
        # consumption order. Slab 0 runs chunks (256,256,512 tokens) as
        # [gate-a, gate-b, up-a, up-b] so x arrives in 64KB half-tiles
        # paced to the p-state-ramped consumption, and u0 is needed ~3.4us
        # after T0 rather than ~1.7us.
        xa = lambda k, h: (xtall[0][:, k, h * 256 : (h + 1) * 256],
                           xt_d[0, :, k, h * 256 : (h + 1) * 256])
        nc.scalar.dma_start(out=g0[:, 0, 0:2, :], in_=gu_d[0, :, 0, 0:2])  # g0k01
        nc.scalar.dma_start(*xa(0, 0))
        nc.sync.dma_start(out=g0[:, 0, 2:4, :], in_=gu_d[0, :, 0, 2:4])  # g0k23
        nc.gpsimd.dma_start(out=g0[:, 1, 0:4, :], in_=gu_d[0, :, 1, 0:4])  # u0a
        nc.sync.dma_start(*xa(1, 0))
        nc.scalar.dma_start(*xa(2, 0))
        nc.sync.dma_start(out=g0[:, 0, 4:6, :], in_=gu_d[0, :, 0, 4:6])  # g0k45
        nc.scalar.dma_start(*xa(3, 0))
        nc.sync.dma_start(*xa(4, 0))
        nc.scalar.dma_start(*xa(5, 0))
        nc.sync.dma_start(out=g0[:, 0, 6:8, :], in_=gu_d[0, :, 0, 6:8])  # g0k67
        nc.scalar.dma_start(*xa(6, 0))
        nc.sync.dma_start(*xa(7, 0))
        nc.scalar.dma_start(*xa(0, 1))
        nc.sync.dma_start(*xa(1, 1))
        nc.scalar.dma_start(*xa(2, 1))
        nc.sync.dma_start(*xa(3, 1))
        nc.scalar.dma_start(*xa(4, 1))
        nc.sync.dma_start(*xa(5, 1))
        nc.scalar.dma_start(out=g0[:, 1, 4:8, :], in_=gu_d[0, :, 1, 4:8])  # u0b
        nc.sync.dma_start(*xa(6, 1))
        nc.sync.dma_start(*xa(7, 1))
        # x chunk 1 (slab-0 chunk c + all later slabs), per-k across queues
        nc.scalar.dma_start(out=xtall[1][:, 0, :], in_=xt_d[1, :, 0])
        nc.sync.dma_start(out=xtall[1][:, 1, :], in_=xt_d[1, :, 1])
        nc.gpsimd.dma_start(out=xtall[1][:, 2, :], in_=xt_d[1, :, 2])
        nc.sync.dma_start(out=xtall[1][:, 3, :], in_=xt_d[1, :, 3])
        nc.scalar.dma_start(out=xtall[1][:, 4, :], in_=xt_d[1, :, 4])
        nc.sync.dma_start(out=xtall[1][:, 5, :], in_=xt_d[1, :, 5])
        nc.gpsimd.dma_start(out=xtall[1][:, 6, :], in_=xt_d[1, :, 6])
        nc.gpsimd.dma_start(out=xtall[1][:, 7, :], in_=xt_d[1, :, 7])

        # per-slab weight DMAs: g half on sync, u half on scalar
        def emit_gu(i):
            gus[i] = gu_pool.tile([128, 2, KC, 128], dt, tag="gu", name=f"gu{i}")
            nc.sync.dma_start(out=gus[i][:, 0], in_=gu_d[i, :, 0])
            nc.scalar.dma_start(out=gus[i][:, 1], in_=gu_d[i, :, 1])

        emit_gu(1)

        dts = [[None] * IS for _ in range(2)]

        def emit_d(hc, i):
            d = d_pool.tile([128, NCH], dt, tag="d", name=f"d{hc}_{i}")
            nc.sync.dma_start(out=d[:], in_=down_d[i, :, ts(hc, NCH)])
            dts[hc][i] = d

        # stage 1
        dq = [(hc, i) for hc in range(2) for i in range(IS)]
        dqi = 0
        hts = []
        for i in range(IS):
            if i + 2 <= IS - 1:
                emit_gu(i + 2)
            if i >= 4:  # down tiles, 3 per slab, off the critical window
                for _ in range(3):
                    if dqi < len(dq):
                        emit_d(*dq[dqi])
                        dqi += 1
            gu = gus[i]
            ht = h_pool.tile([128, TE], dt, tag="h", name=f"h{i}")

            def mm_chunk(pt, gi, tci, c0, cn):
                for k in range(KC):
                    nc.tensor.matmul(
                        pt[:, 0:cn],
                        lhsT=gu[:, gi, k, :],
                        rhs=xtall[tci][:, k, c0 : c0 + cn],
                        start=(k == 0),
                        stop=(k == KC - 1),
                    )

            def swiglu(p1, p2, t0, cn):
                sl = s_pool.tile([128, NCH], f32, tag="s")
                nc.scalar.activation(sl[:, 0:cn], p1[:, 0:cn], silu_fn)
                nc.vector.tensor_mul(ht[:, t0 : t0 + cn], sl[:, 0:cn], p2[:, 0:cn])

            if i == 0:
                # paced start: chunks a,b = 256 tokens of x chunk 0,
                # gates first (ga, gb, ua, ub), then chunk c = x chunk 1
                p1a = ps1.tile([128, NCH], f32, tag="p1")
                p1b = ps1.tile([128, NCH], f32, tag="p1")
                p2a = ps1.tile([128, NCH], f32, tag="p2")
                p2b = ps1.tile([128, NCH], f32, tag="p2")
                mm_chunk(p1a, 0, 0, 0, 256)
                mm_chunk(p1b, 0, 0, 256, 256)
                mm_chunk(p2a, 1, 0, 0, 256)
                mm_chunk(p2b, 1, 0, 256, 256)
                swiglu(p1a, p2a, 0, 256)
                swiglu(p1b, p2b, 256, 256)
                chunks = [(1, 512, NCH)]
            else:
                chunks = [(0, 0, NCH), (1, 512, NCH)]
            for tci, t0, cn in chunks:
                p1 = ps1.tile([128, NCH], f32, tag="p1")
                p2 = ps1.tile([128, NCH], f32, tag="p2")
                mm_chunk(p1, 0, tci, t0 - (512 * tci), cn)
                mm_chunk(p2, 1, tci, t0 - (512 * tci), cn)
                swiglu(p1, p2, t0, cn)
            hts.append(ht)
        while dqi < len(dq):
            emit_d(*dq[dqi])
            dqi += 1

        # stage 2: out[m,hc] = sum_i hT_i[:, m].T @ down_i[:, hc]
        # last tile split in half to shorten the final flush tail
        for hc in range(H // NCH):
            for m in range(TE // 128):
                last = hc == H // NCH - 1 and m == TE // 128 - 1
                parts = (
                    ((0, 256), (256, 128), (384, 128)) if last else ((0, NCH),)
                )
                for c0, cn in parts:
                    po = ps3.tile([128, NCH], f32, tag="po")
                    for i in range(IS):
                        nc.tensor.matmul(
                            po[:, 0:cn],
                            lhsT=hts[i][:, ts(m, 128)],
                            rhs=dts[hc][i][:, c0 : c0 + cn],
                            start=(i == 0),
                            stop=(i == IS - 1),
                        )
                    ob = o_pool.tile([128, cn], dt, tag="o" if cn == NCH else "oh",
                                     bufs=3 if cn == NCH else 2)
                    nc.vector.tensor_copy(ob[:], po[:, 0:cn])
                    nc.scalar.dma_start(
                        out=out_d[ts(m, 128), hc * NCH + c0 : hc * NCH + c0 + cn],
                        in_=ob[:],
                    )

    nc.compile()
    return nc


def _get_nc(dt_tag: str):
    if dt_tag not in _cache:
        _cache[dt_tag] = _build_nc(dt_tag)
    return _cache[dt_tag]


def _to_bf16(a: np.ndarray) -> np.ndarray:
    """Fast float32 -> bfloat16 with round-to-nearest-even."""
    import ml_dtypes

    u = a.view(np.uint32)
    r = ((u >> 16) & 1) + np.uint32(0x7FFF)
    return ((u + r) >> 16).astype(np.uint16).view(ml_dtypes.bfloat16)


def _prep_in_maps(x, gate, up, down, dt_tag: str):
    """Slice per expert and rearrange for contiguous device DMAs."""
    in_maps = []
    for e in range(E):
        xe = x[e * TE : (e + 1) * TE]  # [TE, H]
        # [2(tc), 128(h%128), KC(h//128), 512(t%512)]
        xtp = np.ascontiguousarray(
            xe.T.reshape(KC, 128, 2, NCH).transpose(2, 1, 0, 3)
        )
        # gate/up [H, I] -> [IS, 128(h%128), KC(h//128), 128(i%128)]
        ge = gate[e].reshape(KC, 128, IS, 128).transpose(2, 1, 0, 3)
        ue = up[e].reshape(KC, 128, IS, 128).transpose(2, 1, 0, 3)
        gue = np.ascontiguousarray(np.stack([ge, ue], axis=2))
        de = np.ascontiguousarray(down[e].reshape(IS, 128, H))
        if dt_tag == "bf16":
            xtp, gue, de = (_to_bf16(a) for a in (xtp, gue, de))
        in_maps.append({"xt": xtp, "gu": gue, "down": de})
    return in_maps


def run(inputs: dict, trace: bool = False, tmpdir=None, dt_tag=None):
    """Full-input entry. Returns (output [T,H] f32, BassKernelResults|None)."""
    x = np.asarray(inputs["permuted_local_hidden_states"], dtype=np.float32)
    gate = np.asarray(inputs["grouped_gate_proj"], dtype=np.float32)
    up = np.asarray(inputs["grouped_up_proj"], dtype=np.float32)
    down = np.asarray(inputs["grouped_down_proj"], dtype=np.float32)
    tpe = np.asarray(inputs["tokens_per_expert"]).astype(np.int64)

    if not (x.shape == (T, H) and tpe.shape == (E,) and np.all(tpe == TE)):
        # general ragged fallback (host): correctness-only path
        out = np.empty((x.shape[0], down.shape[2]), dtype=np.float32)
        off = 0
        for e in range(E):
            n = int(tpe[e])
            xe = x[off : off + n]
            o1 = xe @ gate[e]
            o2 = xe @ up[e]
            with np.errstate(over="ignore"):
                hgl = (o1 / (1.0 + np.exp(-o1))) * o2
            out[off : off + n] = hgl @ down[e]
            off += n
        return out, None

    dt_tag = dt_tag or os.environ.get("BASS_MOE_DT", "bf16")
    from concourse.bass_utils import run_bass_kernel_spmd

    nc = _get_nc(dt_tag)
    in_maps = _prep_in_maps(x, gate, up, down, dt_tag)
    res = run_bass_kernel_spmd(
        nc, in_maps, list(range(E)), trace=trace, tmpdir=tmpdir
    )
    out = np.concatenate(
        [np.asarray(res.results[e]["out"], dtype=np.float32) for e in range(E)],
        axis=0,
    )
    return out, res


def kernel(**inputs) -> np.ndarray:
    out, _ = run(inputs, trace=False)
    return out


# revision 13
# speedup vs baseline: 1.0085x; 1.0085x over previous
"""Grouped MoE MLP (SwiGLU) for TRN2, expert-parallel across 8 NeuronCores.

Problem: T=8192 tokens pre-permuted into 8 contiguous expert segments of 1024,
H=1024, I=2816, per-expert weights gate/up [H,I], down [I,H].
    o1 = x @ gate; o2 = x @ up; h = silu(o1)*o2; out = h @ down

Sharding: expert-parallel - core e computes expert e's segment entirely
(zero collectives). Host slices inputs per expert and concatenates outputs.

Device kernel (per core), all matmuls bf16 (rel err ~4e-3, budget 2e-2):
  - Startup is DMA-delivery-bound: queues go live ~8.5us (after the fixed
    ~7us NEFF prologue) at ~130/130/80 GB/s (scalar/sync/gpsimd). Wave-1
    interleaves slab-0 weights + x tiles across all three queues in
    consumption order; the PE p-state ramp (0.65/1.2GHz for the first
    ~3us of a busy streak) softens the early delivery deadlines.
  - stage 1 (one pass, slab period 6.9us): per i-slab, o1T/o2T [128i,512]
    PSUM-accumulate over 8 h-chunks per 512-token chunk; SwiGLU fused
    scalar(silu)+vector(mul) into resident hT bf16 [I, TE].
  - Weight DMAs (g half on sync, u half on scalar) are emitted inside
    slab bodies at prefetch distance 2, so no queue builds a deep
    backlog; down-proj tiles stream on sync 3-per-slab from slab 4.
    PE stalls are poison: any gap drops the PE out of max p-state for
    ~3us of half-rate matmuls. ps1 bufs=3 decouples the SwiGLU readers
    from PSUM slot reuse.
  - stage 2: out[m,hc] = sum_i hT_i[:,m].T @ down_i[:,hc], 22-long PSUM
    chains; output stored bf16 (host upcasts); last tile split in half
    to shorten the final flush tail.
"""

import os
import numpy as np
from contextlib import ExitStack

E, H, I, T = 8, 1024, 2816, 8192
TE = T // E  # tokens per expert = 1024
KC = H // 128  # 8 h-chunks
IS = I // 128  # 22 i-slabs
NCH = 512  # moving free dim per matmul (one PSUM bank of fp32)

_cache: dict = {}


def _build_nc(dt_tag: str):
    from concourse import bacc
    import concourse.tile as tile
    import concourse.mybir as mybir
    from concourse.bass import ts

    f32 = mybir.dt.float32
    dt = {"f32r": mybir.dt.float32r, "bf16": mybir.dt.bfloat16}[dt_tag]

    nc = bacc.Bacc("TRN2", target_bir_lowering=False, debug=False, num_devices=8)
    xt_d = nc.dram_tensor("xt", [2, 128, KC, NCH], dt, kind="ExternalInput").ap()
    gu_d = nc.dram_tensor("gu", [IS, 128, 2, KC, 128], dt, kind="ExternalInput").ap()
    down_d = nc.dram_tensor("down", [IS, 128, H], dt, kind="ExternalInput").ap()
    out_d = nc.dram_tensor("out", [TE, H], dt, kind="ExternalOutput").ap()

    silu_fn = mybir.ActivationFunctionType.Silu

    with tile.TileContext(nc) as tc, ExitStack() as ctx:
        xt_pool = ctx.enter_context(tc.tile_pool(name="xt", bufs=1))
        gu_pool = ctx.enter_context(tc.tile_pool(name="gu", bufs=4))
        h_pool = ctx.enter_context(tc.tile_pool(name="h", bufs=IS))
        d_pool = ctx.enter_context(tc.tile_pool(name="d", bufs=2 * IS))
        s_pool = ctx.enter_context(tc.tile_pool(name="s", bufs=3))
        o_pool = ctx.enter_context(tc.tile_pool(name="o", bufs=3))
        ps1 = ctx.enter_context(tc.tile_pool(name="ps1", bufs=3, space="PSUM"))
        ps3 = ctx.enter_context(tc.tile_pool(name="ps3", bufs=2, space="PSUM"))

        # resident x chunks; gu slabs are double+ buffered (prefetch dist 2)
        xtall = [
            xt_pool.tile([128, KC, NCH], dt, tag=f"xt{tci}", name=f"xt{tci}", bufs=1)
            for tci in range(2)
        ]
        gus = {0: gu_pool.tile([128, 2, KC, 128], dt, tag="gu", name="gu0")}
        g0 = gus[0]

        # ---- wave 1: slab-0 weights + x, interleaved across queues in
        # consumption order (g0 k0..7 with xt0 k0..7, then u0, then xt1).
        # gpsimd's queue is slow (~40-60 GB/s) so it only carries the
        # late-deadline u0 halves and trailing xt1 pieces.
        nc.scalar.dma_start(out=g0[:, 0, 0:2, :], in_=gu_d[0, :, 0, 0:2])  # g0k01
        nc.scalar.dma_start(out=xtall[0][:, 0, :], in_=xt_d[0, :, 0])  # xt00
        nc.sync.dma_start(out=g0[:, 0, 2:4, :], in_=gu_d[0, :, 0, 2:4])  # g0k23
        nc.gpsimd.dma_start(out=g0[:, 1, 0:4, :], in_=gu_d[0, :, 1, 0:4])  # u0a
        nc.scalar.dma_start(out=xtall[0][:, 1, :], in_=xt_d[0, :, 1])  # xt01
        nc.sync.dma_start(out=g0[:, 0, 4:6, :], in_=gu_d[0, :, 0, 4:6])  # g0k45
        nc.scalar.dma_start(out=xtall[0][:, 3, :], in_=xt_d[0, :, 3])  # xt03
        nc.sync.dma_start(out=g0[:, 0, 6:8, :], in_=gu_d[0, :, 0, 6:8])  # g0k67
        nc.sync.dma_start(out=xtall[0][:, 2, :], in_=xt_d[0, :, 2])  # xt02
        nc.scalar.dma_start(out=xtall[0][:, 5, :], in_=xt_d[0, :, 5])  # xt05
        nc.sync.dma_start(out=xtall[0][:, 4, :], in_=xt_d[0, :, 4])  # xt04
        nc.scalar.dma_start(out=xtall[0][:, 7, :], in_=xt_d[0, :, 7])  # xt07
        nc.sync.dma_start(out=xtall[0][:, 6, :], in_=xt_d[0, :, 6])  # xt06
        nc.gpsimd.dma_start(out=g0[:, 1, 4:8, :], in_=gu_d[0, :, 1, 4:8])  # u0b
        # x chunk 1 (needed from ~T0+3.5us), split per-k across queues
        nc.scalar.dma_start(out=xtall[1][:, 0, :], in_=xt_d[1, :, 0])
        nc.sync.dma_start(out=xtall[1][:, 1, :], in_=xt_d[1, :, 1])
        nc.scalar.dma_start(out=xtall[1][:, 2, :], in_=xt_d[1, :, 2])
        nc.sync.dma_start(out=xtall[1][:, 3, :], in_=xt_d[1, :, 3])
        nc.scalar.dma_start(out=xtall[1][:, 4, :], in_=xt_d[1, :, 4])
        nc.gpsimd.dma_start(out=xtall[1][:, 5, :], in_=xt_d[1, :, 5])
        nc.gpsimd.dma_start(out=xtall[1][:, 6, :], in_=xt_d[1, :, 6])
        nc.sync.dma_start(out=xtall[1][:, 7, :], in_=xt_d[1, :, 7])

        # per-slab weight DMAs: g half on sync, u half on scalar
        def emit_gu(i):
            gus[i] = gu_pool.tile([128, 2, KC, 128], dt, tag="gu", name=f"gu{i}")
            nc.sync.dma_start(out=gus[i][:, 0], in_=gu_d[i, :, 0])
            nc.scalar.dma_start(out=gus[i][:, 1], in_=gu_d[i, :, 1])

        emit_gu(1)

        dts = [[None] * IS for _ in range(2)]

        def emit_d(hc, i):
            d = d_pool.tile([128, NCH], dt, tag="d", name=f"d{hc}_{i}")
            nc.sync.dma_start(out=d[:], in_=down_d[i, :, ts(hc, NCH)])
            dts[hc][i] = d

        # stage 1
        dq = [(hc, i) for hc in range(2) for i in range(IS)]
        dqi = 0
        hts = []
        for i in range(IS):
            if i + 2 <= IS - 1:
                emit_gu(i + 2)
            if i >= 4:  # down tiles, 3 per slab, off the critical window
                for _ in range(3):
                    if dqi < len(dq):
                        emit_d(*dq[dqi])
                        dqi += 1
            gu = gus[i]
            ht = h_pool.tile([128, TE], dt, tag="h", name=f"h{i}")
            for tci in range(2):
                p1 = ps1.tile([128, NCH], f32, tag="p1")
                p2 = ps1.tile([128, NCH], f32, tag="p2")
                for k in range(KC):
                    nc.tensor.matmul(
                        p1[:],
                        lhsT=gu[:, 0, k, :],
                        rhs=xtall[tci][:, k, :],
                        start=(k == 0),
                        stop=(k == KC - 1),
                    )
                for k in range(KC):
                    nc.tensor.matmul(
                        p2[:],
                        lhsT=gu[:, 1, k, :],
                        rhs=xtall[tci][:, k, :],
                        start=(k == 0),
                        stop=(k == KC - 1),
                    )
                sl = s_pool.tile([128, NCH], f32, tag="s")
                nc.scalar.activation(sl[:], p1[:], silu_fn)
                nc.vector.tensor_mul(ht[:, ts(tci, NCH)], sl[:], p2[:])
            hts.append(ht)
        while dqi < len(dq):
            emit_d(*dq[dqi])
            dqi += 1

        # stage 2: out[m,hc] = sum_i hT_i[:, m].T @ down_i[:, hc]
        # last tile split in half to shorten the final flush tail
        for hc in range(H // NCH):
            for m in range(TE // 128):
                last = hc == H // NCH - 1 and m == TE // 128 - 1
                parts = (
                    ((0, 256), (256, 128), (384, 128)) if last else ((0, NCH),)
                )
                for c0, cn in parts:
                    po = ps3.tile([128, NCH], f32, tag="po")
                    for i in range(IS):
                        nc.tensor.matmul(
                            po[:, 0:cn],
                            lhsT=hts[i][:, ts(m, 128)],
                            rhs=dts[hc][i][:, c0 : c0 + cn],
                            start=(i == 0),
                            stop=(i == IS - 1),
                        )
                    ob = o_pool.tile([128, cn], dt, tag="o" if cn == NCH else "oh",
                                     bufs=3 if cn == NCH else 2)
                    nc.vector.tensor_copy(ob[:], po[:, 0:cn])
                    nc.scalar.dma_start(
                        out=out_d[ts(m, 128), hc * NCH + c0 : hc * NCH + c0 + cn],
                        in_=ob[:],
                    )

    nc.compile()
    return nc


def _get_nc(dt_tag: str):
    if dt_tag not in _cache:
        _cache[dt_tag] = _build_nc(dt_tag)
    return _cache[dt_tag]


def _to_bf16(a: np.ndarray) -> np.ndarray:
    """Fast float32 -> bfloat16 with round-to-nearest-even."""
    import ml_dtypes

    u = a.view(np.uint32)
    r = ((u >> 16) & 1) + np.uint32(0x7FFF)
    return ((u + r) >> 16).astype(np.uint16).view(ml_dtypes.bfloat16)


def _prep_in_maps(x, gate, up, down, dt_tag: str):
    """Slice per expert and rearrange for contiguous device DMAs."""
    in_maps = []
    for e in range(E):
        xe = x[e * TE : (e + 1) * TE]  # [TE, H]
        # [2(tc), 128(h%128), KC(h//128), 512(t%512)]
        xtp = np.ascontiguousarray(
            xe.T.reshape(KC, 128, 2, NCH).transpose(2, 1, 0, 3)
        )
        # gate/up [H, I] -> [IS, 128(h%128), KC(h//128), 128(i%128)]
        ge = gate[e].reshape(KC, 128, IS, 128).transpose(2, 1, 0, 3)
        ue = up[e].reshape(KC, 128, IS, 128).transpose(2, 1, 0, 3)
        gue = np.ascontiguousarray(np.stack([ge, ue], axis=2))
        de = np.ascontiguousarray(down[e].reshape(IS, 128, H))
        if dt_tag == "bf16":
            xtp, gue, de = (_to_bf16(a) for a in (xtp, gue, de))
        in_maps.append({"xt": xtp, "gu": gue, "down": de})
    return in_maps


def run(inputs: dict, trace: bool = False, tmpdir=None, dt_tag=None):
    """Full-input entry. Returns (output [T,H] f32, BassKernelResults|None)."""
    x = np.asarray(inputs["permuted_local_hidden_states"], dtype=np.float32)
    gate = np.asarray(inputs["grouped_gate_proj"], dtype=np.float32)
    up = np.asarray(inputs["grouped_up_proj"], dtype=np.float32)
    down = np.asarray(inputs["grouped_down_proj"], dtype=np.float32)
    tpe = np.asarray(inputs["tokens_per_expert"]).astype(np.int64)

    if not (x.shape == (T, H) and tpe.shape == (E,) and np.all(tpe == TE)):
        # general ragged fallback (host): correctness-only path
        out = np.empty((x.shape[0], down.shape[2]), dtype=np.float32)
        off = 0
        for e in range(E):
            n = int(tpe[e])
            xe = x[off : off + n]
            o1 = xe @ gate[e]
            o2 = xe @ up[e]
            with np.errstate(over="ignore"):
                hgl = (o1 / (1.0 + np.exp(-o1))) * o2
            out[off : off + n] = hgl @ down[e]
            off += n
        return out, None

    dt_tag = dt_tag or os.environ.get("BASS_MOE_DT", "bf16")
    from concourse.bass_utils import run_bass_kernel_spmd

    nc = _get_nc(dt_tag)
    in_maps = _prep_in_maps(x, gate, up, down, dt_tag)
    res = run_bass_kernel_spmd(
        nc, in_maps, list(range(E)), trace=trace, tmpdir=tmpdir
    )
    out = np.concatenate(
        [np.asarray(res.results[e]["out"], dtype=np.float32) for e in range(E)],
        axis=0,
    )
    return out, res


def kernel(**inputs) -> np.ndarray:
    out, _ = run(inputs, trace=False)
    return out
